# revision 9
# baseline (speedup 1.0000x reference)
"""Causal self-attention for trn2, 8 NeuronCores.

Problem: x[4,2048,1024] @ w_qkv[1024,3072] -> causal MHA (16 heads, d=64)
-> @ w_out[1024,1024].

Sharding: core c handles batch b=c%4 and heads hbase=8*(c//4)..hbase+8
(data parallel on B x tensor parallel on heads). Each core computes the
partial out-projection y_c = att_slice @ w_out[slice]; the host sums the
two partials per batch.

All matmuls run in float32r (PE full-rate fp32, ~1.5e-4 relative) with
operands rounded by their DVE/ACT producers. Softmax denominators come from
a fused ones-column in the AV matmul ([V|1]^T w^T row 64); causal masking
skips above-diagonal tiles entirely and applies one gpsimd affine_select
per diagonal 128x128 block after the exp.

The kernel is a 4-round pipeline over T-quarters: round r transposes x
quarter r (PE transpose-mode), projects qT/kT/V for it, runs attention for
q-block r of every head (which by causality only needs k/V quarters <= r),
and immediately applies the out-projection for those q rows. This keeps
ScalarE (exp) and TensorE overlapped through the whole kernel.
"""

import sys

for p in ("/opt/trn_rl_repo", "/opt/pypackages"):
    if p not in sys.path:
        sys.path.insert(0, p)

import contextlib

import numpy as np

import concourse.bass as bass
import concourse.mybir as mybir
import concourse.tile as tile
from concourse import bacc
from concourse.bass_utils import run_bass_kernel_spmd
from concourse.masks import make_identity

F32 = mybir.dt.float32
F32R = mybir.dt.float32r
EXP = mybir.ActivationFunctionType.Exp

T = 2048          # sequence length
C = 1024          # model dim
HC = 8            # heads per core
D = 64            # head dim
NG = 4            # head-groups of 2 per core
NCT = C // 128    # 8 contraction tiles
NTT = T // 128    # 16 token tiles
SCALE = 0.125     # 1/sqrt(D)


def build_nc():
    nc = bacc.Bacc("TRN2", target_bir_lowering=False, debug=False)

    x_d = nc.dram_tensor("x", [T, C], F32, kind="ExternalInput")
    wq_d = nc.dram_tensor("wq", [C, 512], F32, kind="ExternalInput")
    wk_d = nc.dram_tensor("wk", [C, 512], F32, kind="ExternalInput")
    wv_d = nc.dram_tensor("wv", [C, 512], F32, kind="ExternalInput")
    wo_d = nc.dram_tensor("wo", [512, C], F32, kind="ExternalInput")
    y_d = nc.dram_tensor("y", [T, C], F32, kind="ExternalOutput")

    with tile.TileContext(nc) as tc, contextlib.ExitStack() as ctx:
        persist = ctx.enter_context(tc.tile_pool(name="persist", bufs=1))
        work = ctx.enter_context(tc.tile_pool(name="work", bufs=1))
        ps = ctx.enter_context(tc.tile_pool(name="ps", bufs=1, space="PSUM"))
        dpool = ctx.enter_context(tc.tile_pool(name="dram", bufs=4, space="DRAM"))

        ident = persist.tile([128, 128], F32)
        make_identity(nc, ident)

        kT = [persist.tile([128, T], F32R, tag=f"kT{g}", name=f"kT{g}")
              for g in range(NG)]
        V = persist.tile([128, NTT, HC, 65], F32R, tag="V")

        # out-projection weights, resident from the start
        wo_r = []
        for g in range(NG):
            ws = work.tile([128, C], F32, tag="w_stage", name=f"wos{g}")
            nc.sync.dma_start(out=ws, in_=wo_d.ap()[g * 128:(g + 1) * 128, :])
            wr = persist.tile([128, C], F32R, tag=f"wo{g}", name=f"wo{g}")
            nc.vector.tensor_copy(wr, ws)
            wo_r.append(wr)

        # ones column of V
        ones_f32 = persist.tile([128, NTT, HC], F32, tag="ones")
        nc.vector.memset(ones_f32, 1.0)
        nc.vector.tensor_copy(V[:, :, :, 64], ones_f32)

        # dummy transpose absorbs the gpsimd (make_identity) tick
        dtp = ps.tile([128, 1024], F32, tag="big_a", bufs=2, name="dtp")
        nc.tensor.transpose(dtp[:, 0:128], ident, ident)
        ddum = work.tile([128, 128], F32, tag="tmpB", bufs=2, name="ddum")
        nc.vector.tensor_copy(ddum, dtp[:, 0:128])

        for rnd in range(4):
            q0 = rnd * 512  # first token of this quarter

            # ---- transpose x quarter -> xTq (f32r) ----
            xTq = [work.tile([128, 512], F32R, tag=f"xTq{ct}", name=f"xTq{ct}")
                   for ct in range(NCT)]
            for j in range(4):
                row0 = q0 + j * 128
                x_nat = work.tile([128, C], F32, tag="x_nat", bufs=2)
                nc.sync.dma_start(out=x_nat, in_=x_d.ap()[row0:row0 + 128, :])
                tp = ps.tile([128, 1024], F32, tag="big_a", bufs=2, name="tp")
                for ct in range(NCT):
                    nc.tensor.transpose(
                        tp[:, ct * 128:ct * 128 + 128],
                        x_nat[:, ct * 128:(ct + 1) * 128],
                        ident,
                    )
                for ct in range(NCT):
                    nc.vector.tensor_copy(
                        xTq[ct][:, j * 128:(j + 1) * 128],
                        tp[:, ct * 128:ct * 128 + 128],
                    )

            # ---- qT/kT for this quarter ----
            qTq = []
            for g in range(NG):
                wsq = work.tile([128, NCT, 128], F32, tag="w_stage", name="wsq")
                nc.sync.dma_start(
                    out=wsq,
                    in_=wq_d.ap()[:, g * 128:(g + 1) * 128].rearrange(
                        "(ct p) m -> p ct m", p=128
                    ),
                )
                wrq = work.tile([128, NCT, 128], F32R, tag="w_r", bufs=2, name="wrq")
                nc.vector.tensor_copy(wrq, wsq)
                wsk = work.tile([128, NCT, 128], F32, tag="w_stage", name="wsk")
                nc.sync.dma_start(
                    out=wsk,
                    in_=wk_d.ap()[:, g * 128:(g + 1) * 128].rearrange(
                        "(ct p) m -> p ct m", p=128
                    ),
                )
                wrk = work.tile([128, NCT, 128], F32R, tag="w_r", bufs=2, name="wrk")
                nc.vector.tensor_copy(wrk, wsk)

                pqk = ps.tile([128, 1024], F32, tag="big_a", bufs=2, name="pqk")
                for ct in range(NCT):
                    nc.tensor.matmul(
                        pqk[:, 0:512], wrq[:, ct, :], xTq[ct],
                        start=(ct == 0), stop=(ct == NCT - 1),
                    )
                    nc.tensor.matmul(
                        pqk[:, 512:1024], wrk[:, ct, :], xTq[ct],
                        start=(ct == 0), stop=(ct == NCT - 1),
                    )
                qq = work.tile([128, 512], F32R, tag=f"qTq{g}", name=f"qTq{g}")
                nc.vector.tensor_copy(qq, pqk[:, 0:512])
                qTq.append(qq)
                nc.vector.tensor_copy(kT[g][:, q0:q0 + 512], pqk[:, 512:1024])

            # ---- V for this quarter (ct-outer, wv streamed) ----
            pv0 = ps.tile([128, 1024], F32, tag="big_a", bufs=2, name="pv0")
            pv1 = ps.tile([128, 1024], F32, tag="big_a", bufs=2, name="pv1")
            pvs = (pv0, pv1)
            for ct in range(NCT):
                wvs = work.tile([128, 512], F32, tag="wv_s", bufs=2, name="wvs")
                nc.sync.dma_start(
                    out=wvs, in_=wv_d.ap()[ct * 128:(ct + 1) * 128, :]
                )
                wvr = work.tile([128, 512], F32R, tag="wv_r", bufs=2, name="wvr")
                nc.vector.tensor_copy(wvr, wvs)
                for jl in range(4):
                    nc.tensor.matmul(
                        pvs[jl // 2][:, (jl % 2) * 512:(jl % 2) * 512 + 512],
                        xTq[ct][:, jl * 128:(jl + 1) * 128],
                        wvr,
                        start=(ct == 0), stop=(ct == NCT - 1),
                    )
            for jl in range(4):
                tt = rnd * 4 + jl
                src = pvs[jl // 2][:, (jl % 2) * 512:(jl % 2) * 512 + 512]
                for h in range(HC):
                    nc.vector.tensor_copy(
                        V[:, tt, h, 0:64], src[:, h * 64:(h + 1) * 64]
                    )

            # ---- attention: q-block rnd for every group ----
            qb = rnd
            nkt = 4 * (qb + 1)
            attTq = []
            for g in range(NG):
                hA, hB = 2 * g, 2 * g + 1
                av_A = ps.tile([65, 512], F32, tag="av_A", name="av_A")
                av_B = ps.tile([65, 512], F32, tag="av_B", name="av_B")
                for kp in range(nkt // 2):
                    sA = ps.tile([128, 1024], F32, tag="big_a", bufs=2, name="sA")
                    sB = ps.tile([128, 1024], F32, tag="big_b", name="sB")
                    for i in range(2):
                        kt = 2 * kp + i
                        nc.tensor.matmul(
                            sA[:, i * 512:(i + 1) * 512],
                            kT[g][0:64, kt * 128:(kt + 1) * 128],
                            qTq[g][0:64, :],
                            start=True, stop=True,
                            tile_position=(0, 0),
                        )
                        nc.tensor.matmul(
                            sB[:, i * 512:(i + 1) * 512],
                            kT[g][64:128, kt * 128:(kt + 1) * 128],
                            qTq[g][64:128, :],
                            start=True, stop=True,
                            tile_position=(64, 0),
                        )
                    wT_A = work.tile([128, 1024], F32R, tag="wT_A", bufs=2)
                    wT_B = work.tile([128, 1024], F32R, tag="wT_B", bufs=2)
                    nc.scalar.activation(wT_A, sA, EXP, scale=SCALE)
                    nc.scalar.activation(wT_B, sB, EXP, scale=SCALE)
                    for i in range(2):
                        kt = 2 * kp + i
                        j = kt - 4 * qb
                        if j >= 0:  # diagonal supertile: causal select
                            ncols = 128 * j + 128
                            for wT in (wT_A, wT_B):
                                nc.gpsimd.affine_select(
                                    out=wT[:, i * 512:i * 512 + ncols],
                                    in_=wT[:, i * 512:i * 512 + ncols],
                                    compare_op=mybir.AluOpType.is_ge,
                                    fill=0.0,
                                    base=-128 * j,
                                    pattern=[[1, ncols]],
                                    channel_multiplier=-1,
                                )
                    for i in range(2):
                        kt = 2 * kp + i
                        nc.tensor.matmul(
                            av_A, V[:, kt, hA, :], wT_A[:, i * 512:(i + 1) * 512],
                            start=(kt == 0), stop=(kt == nkt - 1),
                        )
                        nc.tensor.matmul(
                            av_B, V[:, kt, hB, :], wT_B[:, i * 512:(i + 1) * 512],
                            start=(kt == 0), stop=(kt == nkt - 1),
                        )
                # move accumulators off PSUM fast, then normalize off the
                # critical path: recip row 64, DRAM-bounce broadcast, multiply
                avc = work.tile([65, 1024], F32, tag="avc", name="avc")
                nc.vector.tensor_copy(avc[:, 0:512], av_A)
                nc.vector.tensor_copy(avc[:, 512:1024], av_B)
                rec = work.tile([65, 1024], F32, tag="rec", name="rec")
                nc.vector.reciprocal(rec[64:65, :], avc[64:65, :])
                rec_d = dpool.tile([1, 1024], F32, tag="rec_d", name="rec_d")
                nc.sync.dma_start(out=rec_d, in_=rec[64:65, :])
                rep = work.tile([64, 1024], F32, tag="rep", name="rep")
                nc.sync.dma_start(
                    out=rep,
                    in_=bass.AP(rec_d.tensor, rec_d.offset, [[0, 64], [1, 1024]]),
                )
                att = work.tile([128, 512], F32R, tag=f"attTq{g}", name=f"attTq{g}")
                nc.vector.tensor_mul(att[0:64, :], avc[0:64, 0:512], rep[:, 0:512])
                tmpB = work.tile([64, 512], F32R, tag="tmpB", bufs=2, name="tmpB")
                nc.vector.tensor_mul(tmpB, avc[0:64, 512:1024], rep[:, 512:1024])
                nc.sync.dma_start(out=att[64:128, :], in_=tmpB)
                attTq.append(att)

            # ---- out projection for this quarter's q rows ----
            for qtl in range(4):
                qt = rnd * 4 + qtl
                psy = ps.tile([128, 1024], F32, tag="big_b", name="psy")
                for g in range(NG):
                    for half in range(2):
                        nc.tensor.matmul(
                            psy[:, half * 512:(half + 1) * 512],
                            attTq[g][:, qtl * 128:(qtl + 1) * 128],
                            wo_r[g][:, half * 512:(half + 1) * 512],
                            start=(g == 0),
                            stop=(g == NG - 1),
                        )
                y_sb = work.tile([128, C], F32, tag="y_sb", name="y_sb")
                nc.vector.tensor_copy(y_sb, psy)
                nc.sync.dma_start(
                    out=y_d.ap()[qt * 128:(qt + 1) * 128, :], in_=y_sb
                )

    nc.compile()
    return nc


_NC_CACHE = None


def _get_nc():
    global _NC_CACHE
    if _NC_CACHE is None:
        _NC_CACHE = build_nc()
    return _NC_CACHE


def kernel(x, w_qkv, w_out, _trace=False):
    B = x.shape[0]
    x = np.ascontiguousarray(x, dtype=np.float32)
    w_qkv = np.ascontiguousarray(w_qkv, dtype=np.float32)
    w_out = np.ascontiguousarray(w_out, dtype=np.float32)

    nc = _get_nc()
    in_maps = []
    for core in range(8):
        b = core % B
        hbase = (core // B) * HC
        lo, hi = hbase * D, hbase * D + HC * D
        in_maps.append({
            "x": x[b],
            "wq": np.ascontiguousarray(w_qkv[:, lo:hi]),
            "wk": np.ascontiguousarray(w_qkv[:, C + lo:C + hi]),
            "wv": np.ascontiguousarray(w_qkv[:, 2 * C + lo:2 * C + hi]),
            "wo": np.ascontiguousarray(w_out[lo:hi, :]),
        })

    res = run_bass_kernel_spmd(nc, in_maps, core_ids=list(range(8)), trace=_trace)
    ys = [r["y"] for r in res.results]
    out = np.empty((B, T, C), dtype=np.float32)
    for b in range(B):
        out[b] = ys[b] + ys[b + B]
    if _trace:
        return out, res
    return out


# revision 10
# speedup vs baseline: 1.0093x; 1.0093x over previous
"""Causal self-attention for trn2, 8 NeuronCores.

Problem: x[4,2048,1024] @ w_qkv[1024,3072] -> causal MHA (16 heads, d=64)
-> @ w_out[1024,1024].

Sharding: core c handles batch b=c%4 and heads hbase=8*(c//4)..hbase+8
(data parallel on B x tensor parallel on heads). Each core computes the
partial out-projection y_c = att_slice @ w_out[slice]; the host sums the
two partials per batch.

v3: all matmul operands in bf16 (fp32 PSUM accumulation), which enables
fast-weight-load and full-rate matmuls; x is transposed by casting to a
bf16 DRAM scratch once (SWDGE cast-DMA) and then using hardware
DMA-transpose loads. Softmax denominators come from a fused ones-column in
the AV matmul ([V|1]^T w^T row 64); reciprocals are computed on ScalarE as
exp(-ln(d)) (both functions in one ACT table set). Causal masking skips
above-diagonal tiles and applies one gpsimd affine_select per diagonal
128x128 block after the exp.

The kernel is a 4-round pipeline over T-quarters: round r loads/transposes
x quarter r, projects qT/kT/V for it, runs attention for q-block r of every
head (causality: only needs k/V quarters <= r), then immediately applies
the out-projection for those q rows.
"""

import sys

for p in ("/opt/trn_rl_repo", "/opt/pypackages"):
    if p not in sys.path:
        sys.path.insert(0, p)

import contextlib

import numpy as np

import concourse.bass as bass
import concourse.mybir as mybir
import concourse.tile as tile
from concourse import bacc
from concourse.bass_utils import run_bass_kernel_spmd

F32 = mybir.dt.float32
BF = mybir.dt.bfloat16
EXP = mybir.ActivationFunctionType.Exp
LN = mybir.ActivationFunctionType.Ln

T = 2048          # sequence length
C = 1024          # model dim
HC = 8            # heads per core
D = 64            # head dim
NG = 4            # head-groups of 2 per core
NCT = C // 128    # 8 contraction tiles
NTT = T // 128    # 16 token tiles
SCALE = 0.125     # 1/sqrt(D)


def build_nc():
    nc = bacc.Bacc("TRN2", target_bir_lowering=False, debug=False)

    x_d = nc.dram_tensor("x", [T, C], F32, kind="ExternalInput")
    wq_d = nc.dram_tensor("wq", [C, 512], F32, kind="ExternalInput")
    wk_d = nc.dram_tensor("wk", [C, 512], F32, kind="ExternalInput")
    wv_d = nc.dram_tensor("wv", [C, 512], F32, kind="ExternalInput")
    wo_d = nc.dram_tensor("wo", [512, C], F32, kind="ExternalInput")
    y_d = nc.dram_tensor("y", [T, C], F32, kind="ExternalOutput")

    with tile.TileContext(nc) as tc, contextlib.ExitStack() as ctx:
        persist = ctx.enter_context(tc.tile_pool(name="persist", bufs=1))
        work = ctx.enter_context(tc.tile_pool(name="work", bufs=1))
        ps = ctx.enter_context(tc.tile_pool(name="ps", bufs=1, space="PSUM"))
        dpool = ctx.enter_context(tc.tile_pool(name="dram", bufs=1, space="DRAM"))

        kT = [persist.tile([128, T], BF, tag=f"kT{g}", name=f"kT{g}")
              for g in range(NG)]
        V = persist.tile([128, NTT, HC, 65], BF, tag="V")

        # x -> bf16 DRAM scratch (cast during SWDGE DMA), quarter by quarter
        # so the per-round transpose loads can start before the whole cast
        # is done.
        xbf = dpool.tile([T, C], BF, tag="xbf", name="xbf")
        for rnd in range(4):
            nc.gpsimd.dma_start(
                out=xbf[rnd * 512:(rnd + 1) * 512, :],
                in_=x_d.ap()[rnd * 512:(rnd + 1) * 512, :],
            )

        # out-projection weights, resident from the start
        wo_r = []
        for g in range(NG):
            ws = work.tile([128, C], F32, tag="w_stage", name=f"wos{g}")
            nc.sync.dma_start(out=ws, in_=wo_d.ap()[g * 128:(g + 1) * 128, :])
            wr = persist.tile([128, C], BF, tag=f"wo{g}", name=f"wo{g}")
            nc.vector.tensor_copy(wr, ws)
            wo_r.append(wr)

        # ones column of V
        ones_f32 = persist.tile([128, NTT, HC], F32, tag="ones")
        nc.vector.memset(ones_f32, 1.0)
        nc.vector.tensor_copy(V[:, :, :, 64], ones_f32)

        for rnd in range(4):
            q0 = rnd * 512  # first token of this quarter

            # ---- xT quarter via hardware DMA-transpose ----
            xTq = [work.tile([128, 512], BF, tag=f"xTq{ct}", name=f"xTq{ct}",
                             bufs=2)
                   for ct in range(NCT)]
            for ct in range(NCT):
                nc.sync.dma_start_transpose(
                    out=xTq[ct],
                    in_=xbf[q0:q0 + 512, ct * 128:(ct + 1) * 128],
                )

            # ---- qT/kT for this quarter ----
            qTq = []
            for g in range(NG):
                wsq = work.tile([128, NCT, 128], F32, tag="w_stage", name="wsq")
                nc.sync.dma_start(
                    out=wsq,
                    in_=wq_d.ap()[:, g * 128:(g + 1) * 128].rearrange(
                        "(ct p) m -> p ct m", p=128
                    ),
                )
                wrq = work.tile([128, NCT, 128], BF, tag="w_r", bufs=2, name="wrq")
                nc.vector.tensor_copy(wrq, wsq)
                wsk = work.tile([128, NCT, 128], F32, tag="w_stage", name="wsk")
                nc.sync.dma_start(
                    out=wsk,
                    in_=wk_d.ap()[:, g * 128:(g + 1) * 128].rearrange(
                        "(ct p) m -> p ct m", p=128
                    ),
                )
                wrk = work.tile([128, NCT, 128], BF, tag="w_r", bufs=2, name="wrk")
                nc.vector.tensor_copy(wrk, wsk)

                pqk = ps.tile([128, 1024], F32, tag="big_a", bufs=2, name="pqk")
                for ct in range(NCT):
                    nc.tensor.matmul(
                        pqk[:, 0:512], wrq[:, ct, :], xTq[ct],
                        start=(ct == 0), stop=(ct == NCT - 1),
                    )
                    nc.tensor.matmul(
                        pqk[:, 512:1024], wrk[:, ct, :], xTq[ct],
                        start=(ct == 0), stop=(ct == NCT - 1),
                    )
                qq = work.tile([128, 512], BF, tag=f"qTq{g}", name=f"qTq{g}")
                nc.vector.tensor_copy(qq, pqk[:, 0:512])
                qTq.append(qq)
                nc.vector.tensor_copy(kT[g][:, q0:q0 + 512], pqk[:, 512:1024])

            # ---- V for this quarter (ct-outer, wv streamed) ----
            pv0 = ps.tile([128, 1024], F32, tag="big_a", bufs=2, name="pv0")
            pv1 = ps.tile([128, 1024], F32, tag="big_a", bufs=2, name="pv1")
            pvs = (pv0, pv1)
            for ct in range(NCT):
                wvs = work.tile([128, 512], F32, tag="wv_s", bufs=2, name="wvs")
                nc.sync.dma_start(
                    out=wvs, in_=wv_d.ap()[ct * 128:(ct + 1) * 128, :]
                )
                wvr = work.tile([128, 512], BF, tag="wv_r", bufs=2, name="wvr")
                nc.vector.tensor_copy(wvr, wvs)
                for jl in range(4):
                    nc.tensor.matmul(
                        pvs[jl // 2][:, (jl % 2) * 512:(jl % 2) * 512 + 512],
                        xTq[ct][:, jl * 128:(jl + 1) * 128],
                        wvr,
                        start=(ct == 0), stop=(ct == NCT - 1),
                    )
            for jl in range(4):
                tt = rnd * 4 + jl
                src = pvs[jl // 2][:, (jl % 2) * 512:(jl % 2) * 512 + 512]
                for h in range(HC):
                    nc.vector.tensor_copy(
                        V[:, tt, h, 0:64], src[:, h * 64:(h + 1) * 64]
                    )

            # ---- attention: q-block rnd for every group ----
            qb = rnd
            nkt = 4 * (qb + 1)
            attTq = []
            for g in range(NG):
                hA, hB = 2 * g, 2 * g + 1
                av_A = ps.tile([65, 512], F32, tag="av_A", name="av_A")
                av_B = ps.tile([65, 512], F32, tag="av_B", name="av_B")
                for kp in range(nkt // 2):
                    sA = ps.tile([128, 1024], F32, tag="big_a", bufs=2, name="sA")
                    sB = ps.tile([128, 1024], F32, tag="big_b", name="sB")
                    for i in range(2):
                        kt = 2 * kp + i
                        nc.tensor.matmul(
                            sA[:, i * 512:(i + 1) * 512],
                            kT[g][0:64, kt * 128:(kt + 1) * 128],
                            qTq[g][0:64, :],
                            start=True, stop=True,
                            tile_position=(0, 0),
                        )
                        nc.tensor.matmul(
                            sB[:, i * 512:(i + 1) * 512],
                            kT[g][64:128, kt * 128:(kt + 1) * 128],
                            qTq[g][64:128, :],
                            start=True, stop=True,
                            tile_position=(64, 0),
                        )
                    wT_A = work.tile([128, 1024], BF, tag="wT_A", bufs=3)
                    wT_B = work.tile([128, 1024], BF, tag="wT_B", bufs=3)
                    nc.scalar.activation(wT_A, sA, EXP, scale=SCALE)
                    nc.scalar.activation(wT_B, sB, EXP, scale=SCALE)
                    for i in range(2):
                        kt = 2 * kp + i
                        j = kt - 4 * qb
                        if j >= 0:  # diagonal supertile: causal select
                            ncols = 128 * j + 128
                            for wT in (wT_A, wT_B):
                                nc.gpsimd.affine_select(
                                    out=wT[:, i * 512:i * 512 + ncols],
                                    in_=wT[:, i * 512:i * 512 + ncols],
                                    compare_op=mybir.AluOpType.is_ge,
                                    fill=0.0,
                                    base=-128 * j,
                                    pattern=[[1, ncols]],
                                    channel_multiplier=-1,
                                )
                    for i in range(2):
                        kt = 2 * kp + i
                        nc.tensor.matmul(
                            av_A, V[:, kt, hA, :], wT_A[:, i * 512:(i + 1) * 512],
                            start=(kt == 0), stop=(kt == nkt - 1),
                        )
                        nc.tensor.matmul(
                            av_B, V[:, kt, hB, :], wT_B[:, i * 512:(i + 1) * 512],
                            start=(kt == 0), stop=(kt == nkt - 1),
                        )
                # move accumulators off PSUM fast, then normalize off the
                # critical path: 1/d = exp(-ln(d)) on ScalarE, DRAM-bounce
                # partition broadcast, multiply
                avc = work.tile([65, 1024], F32, tag="avc", bufs=2, name="avc")
                nc.vector.tensor_copy(avc[:, 0:512], av_A)
                nc.vector.tensor_copy(avc[:, 512:1024], av_B)
                rec = work.tile([65, 1024], F32, tag="rec", bufs=2, name="rec")
                nc.scalar.activation(rec[64:65, :], avc[64:65, :], LN)
                nc.scalar.activation(rec[64:65, :], rec[64:65, :], EXP, scale=-1.0)
                rec_d = dpool.tile([1, 1024], F32, tag="rec_d", bufs=4, name="rec_d")
                nc.sync.dma_start(out=rec_d, in_=rec[64:65, :])
                rep = work.tile([64, 1024], F32, tag="rep", bufs=2, name="rep")
                nc.sync.dma_start(
                    out=rep,
                    in_=bass.AP(rec_d.tensor, rec_d.offset, [[0, 64], [1, 1024]]),
                )
                att = work.tile([128, 512], BF, tag=f"attTq{g}", bufs=2,
                                name=f"attTq{g}")
                nc.vector.tensor_mul(att[0:64, :], avc[0:64, 0:512], rep[:, 0:512])
                tmpB = work.tile([64, 512], BF, tag="tmpB", bufs=2, name="tmpB")
                nc.vector.tensor_mul(tmpB, avc[0:64, 512:1024], rep[:, 512:1024])
                nc.sync.dma_start(out=att[64:128, :], in_=tmpB)
                attTq.append(att)

            # ---- out projection for this quarter's q rows ----
            for qtl in range(4):
                qt = rnd * 4 + qtl
                psy = ps.tile([128, 1024], F32, tag="big_b", name="psy")
                for g in range(NG):
                    for half in range(2):
                        nc.tensor.matmul(
                            psy[:, half * 512:(half + 1) * 512],
                            attTq[g][:, qtl * 128:(qtl + 1) * 128],
                            wo_r[g][:, half * 512:(half + 1) * 512],
                            start=(g == 0),
                            stop=(g == NG - 1),
                        )
                y_sb = work.tile([128, C], F32, tag="y_sb", bufs=2, name="y_sb")
                nc.vector.tensor_copy(y_sb, psy)
                nc.sync.dma_start(
                    out=y_d.ap()[qt * 128:(qt + 1) * 128, :], in_=y_sb
                )

    nc.compile()
    return nc


_NC_CACHE = None


def _get_nc():
    global _NC_CACHE
    if _NC_CACHE is None:
        _NC_CACHE = build_nc()
    return _NC_CACHE


def kernel(x, w_qkv, w_out, _trace=False):
    B = x.shape[0]
    x = np.ascontiguousarray(x, dtype=np.float32)
    w_qkv = np.ascontiguousarray(w_qkv, dtype=np.float32)
    w_out = np.ascontiguousarray(w_out, dtype=np.float32)

    nc = _get_nc()
    in_maps = []
    for core in range(8):
        b = core % B
        hbase = (core // B) * HC
        lo, hi = hbase * D, hbase * D + HC * D
        in_maps.append({
            "x": x[b],
            "wq": np.ascontiguousarray(w_qkv[:, lo:hi]),
            "wk": np.ascontiguousarray(w_qkv[:, C + lo:C + hi]),
            "wv": np.ascontiguousarray(w_qkv[:, 2 * C + lo:2 * C + hi]),
            "wo": np.ascontiguousarray(w_out[lo:hi, :]),
        })

    res = run_bass_kernel_spmd(nc, in_maps, core_ids=list(range(8)), trace=_trace)
    ys = [r["y"] for r in res.results]
    out = np.empty((B, T, C), dtype=np.float32)
    for b in range(B):
        out[b] = ys[b] + ys[b + B]
    if _trace:
        return out, res
    return out


# revision 16
# speedup vs baseline: 1.1026x; 1.0924x over previous
"""Causal self-attention for trn2, 8 NeuronCores.

Problem: x[4,2048,1024] @ w_qkv[1024,3072] -> causal MHA (16 heads, d=64)
-> @ w_out[1024,1024].

Sharding: core c handles batch b=c%4 and heads hbase=8*(c//4)..hbase+8
(data parallel on B x tensor parallel on heads). Each core computes the
partial out-projection y_c = att_slice @ w_out[slice]; the host sums the
two partials per batch.

v4: all matmul operands bf16 (fp32 PSUM accumulation). x is cast to a
ct-major bf16 DRAM scratch (SWDGE cast-DMA, contiguous [2048,128] blocks)
and transposed with hardware DMA-transpose loads. All weights are cast
once into resident bf16 tiles by SWDGE cast-DMAs. Softmax denominators
come from a fused ones-column in the AV matmul ([V|1]^T w^T row 64);
causal masking skips above-diagonal tiles and applies one gpsimd
affine_select per diagonal 128x128 block after the exp. Normalization:
DVE reciprocal + DRAM-bounce partition broadcast + multiply, staged off
PSUM so nothing blocks the accumulators.

4-round pipeline over T-quarters: round r transposes quarter r, projects
qT/kT/V for it, runs attention q-block r for every head (causality needs
only k/V quarters <= r), then the out-projection for those q rows. PSUM:
sA/sB double-buffered [128,512] scores, av_A/av_B accumulators, and a
dedicated [128,1024] projection tag so next-round projection matmuls can
fill TensorE gaps while ScalarE paces the attention exps.
"""

import sys

for p in ("/opt/trn_rl_repo", "/opt/pypackages"):
    if p not in sys.path:
        sys.path.insert(0, p)

import contextlib

import numpy as np

import concourse.bass as bass
import concourse.mybir as mybir
import concourse.tile as tile
from concourse import bacc
from concourse.bass_utils import run_bass_kernel_spmd

F32 = mybir.dt.float32
BF = mybir.dt.bfloat16
EXP = mybir.ActivationFunctionType.Exp

T = 2048          # sequence length
C = 1024          # model dim
HC = 8            # heads per core
D = 64            # head dim
NG = 4            # head-groups of 2 per core
NCT = C // 128    # 8 contraction tiles
NTT = T // 128    # 16 token tiles
SCALE = 0.125     # 1/sqrt(D)


def build_nc():
    nc = bacc.Bacc("TRN2", target_bir_lowering=False, debug=False)

    x_d = nc.dram_tensor("x", [T, C], F32, kind="ExternalInput")
    wq_d = nc.dram_tensor("wq", [C, 512], F32, kind="ExternalInput")
    wk_d = nc.dram_tensor("wk", [C, 512], F32, kind="ExternalInput")
    wv_d = nc.dram_tensor("wv", [C, 512], F32, kind="ExternalInput")
    wo_d = nc.dram_tensor("wo", [512, C], F32, kind="ExternalInput")
    y_d = nc.dram_tensor("y", [T, C], F32, kind="ExternalOutput")

    with tile.TileContext(nc) as tc, contextlib.ExitStack() as ctx:
        persist = ctx.enter_context(tc.tile_pool(name="persist", bufs=1))
        work = ctx.enter_context(tc.tile_pool(name="work", bufs=1))
        ps = ctx.enter_context(tc.tile_pool(name="ps", bufs=1, space="PSUM"))
        dpool = ctx.enter_context(tc.tile_pool(name="dram", bufs=1, space="DRAM"))

        kT = [persist.tile([128, T], BF, tag=f"kT{g}", name=f"kT{g}")
              for g in range(NG)]
        V = persist.tile([128, NTT, HC, 65], BF, tag="V")

        # x -> bf16 DRAM scratch. The cast must be a CONTIGUOUS SWDGE DMA:
        # strided cast-DMAs truncate instead of round-to-nearest, and the
        # truncation bias blows up the dot products downstream.
        xbf = dpool.tile([T, C], BF, tag="xbf", name="xbf")
        for rnd in range(4):
            nc.gpsimd.dma_start(
                out=xbf[rnd * 512:(rnd + 1) * 512, :],
                in_=x_d.ap()[rnd * 512:(rnd + 1) * 512, :],
            )

        # resident bf16 weights: contiguous SWDGE cast to DRAM (exact RNE),
        # then plain rearranged bf16 loads
        wqd_bf = dpool.tile([C, 512], BF, tag="wqd_bf", name="wqd_bf")
        nc.gpsimd.dma_start(out=wqd_bf, in_=wq_d.ap())
        wkd_bf = dpool.tile([C, 512], BF, tag="wkd_bf", name="wkd_bf")
        nc.gpsimd.dma_start(out=wkd_bf, in_=wk_d.ap())
        wvd_bf = dpool.tile([C, 512], BF, tag="wvd_bf", name="wvd_bf")
        nc.gpsimd.dma_start(out=wvd_bf, in_=wv_d.ap())
        wod_bf = dpool.tile([512, C], BF, tag="wod_bf", name="wod_bf")
        nc.gpsimd.dma_start(out=wod_bf, in_=wo_d.ap())
        wq_bf = persist.tile([128, NCT, 512], BF, tag="wq_bf")
        nc.sync.dma_start(
            out=wq_bf, in_=wqd_bf.rearrange("(ct p) m -> p ct m", p=128))
        wk_bf = persist.tile([128, NCT, 512], BF, tag="wk_bf")
        nc.sync.dma_start(
            out=wk_bf, in_=wkd_bf.rearrange("(ct p) m -> p ct m", p=128))
        wv_bf = persist.tile([128, NCT, 512], BF, tag="wv_bf")
        nc.sync.dma_start(
            out=wv_bf, in_=wvd_bf.rearrange("(ct p) m -> p ct m", p=128))
        wo_bf = persist.tile([128, NG, C], BF, tag="wo_bf")
        nc.sync.dma_start(
            out=wo_bf, in_=wod_bf.rearrange("(g p) c -> p g c", p=128))

        # ones column of V
        ones_f32 = persist.tile([128, NTT, HC], F32, tag="ones")
        nc.vector.memset(ones_f32, 1.0)
        nc.vector.tensor_copy(V[:, :, :, 64], ones_f32)

        for rnd in range(4):
            q0 = rnd * 512  # first token of this quarter

            # ---- xT quarter via hardware DMA-transpose (2 HWDGE rings) ----
            xTq = [work.tile([128, 512], BF, tag=f"xTq{ct}", name=f"xTq{ct}",
                             bufs=2)
                   for ct in range(NCT)]
            for ct in range(NCT):
                nc.sync.dma_start_transpose(
                    out=xTq[ct], in_=xbf[q0:q0 + 512, ct * 128:(ct + 1) * 128]
                )

            # ---- qT/kT for this quarter ----
            qTq = []
            for g in range(NG):
                pqk = ps.tile([128, 1024], F32, tag="pp", name="pqk")
                for ct in range(NCT):
                    nc.tensor.matmul(
                        pqk[:, 0:512],
                        wq_bf[:, ct, g * 128:(g + 1) * 128],
                        xTq[ct],
                        start=(ct == 0), stop=(ct == NCT - 1),
                    )
                    nc.tensor.matmul(
                        pqk[:, 512:1024],
                        wk_bf[:, ct, g * 128:(g + 1) * 128],
                        xTq[ct],
                        start=(ct == 0), stop=(ct == NCT - 1),
                    )
                qq = work.tile([128, 512], BF, tag=f"qTq{g}", bufs=2,
                               name=f"qTq{g}")
                nc.vector.tensor_copy(qq, pqk[:, 0:512])
                qTq.append(qq)
                nc.vector.tensor_copy(kT[g][:, q0:q0 + 512], pqk[:, 512:1024])

            # ---- V for this quarter (two tt-pairs per psum tile) ----
            for half in range(2):
                pv = ps.tile([128, 1024], F32, tag="pp", name="pv")
                for ct in range(NCT):
                    for sub in range(2):
                        jl = half * 2 + sub
                        nc.tensor.matmul(
                            pv[:, sub * 512:(sub + 1) * 512],
                            xTq[ct][:, jl * 128:(jl + 1) * 128],
                            wv_bf[:, ct, :],
                            start=(ct == 0), stop=(ct == NCT - 1),
                        )
                for sub in range(2):
                    tt = rnd * 4 + half * 2 + sub
                    for h in range(HC):
                        nc.vector.tensor_copy(
                            V[:, tt, h, 0:64],
                            pv[:, sub * 512 + h * 64: sub * 512 + h * 64 + 64],
                        )

            # ---- attention: q-block rnd for every group ----
            qb = rnd
            nkt = 4 * (qb + 1)
            attTq = []
            for g in range(NG):
                hA, hB = 2 * g, 2 * g + 1
                av_A = ps.tile([65, 512], F32, tag="av_A", name="av_A")
                av_B = ps.tile([65, 512], F32, tag="av_B", name="av_B")
                for kt in range(nkt):
                    sAB = ps.tile([128, 1024], F32, tag="sAB", bufs=2, name="sAB")
                    nc.tensor.matmul(
                        sAB[:, 0:512],
                        kT[g][0:64, kt * 128:(kt + 1) * 128],
                        qTq[g][0:64, :],
                        start=True, stop=True,
                        tile_position=(0, 0),
                    )
                    nc.tensor.matmul(
                        sAB[:, 512:1024],
                        kT[g][64:128, kt * 128:(kt + 1) * 128],
                        qTq[g][64:128, :],
                        start=True, stop=True,
                        tile_position=(64, 0),
                    )
                    wT_A = work.tile([128, 512], BF, tag="wT_A", bufs=4)
                    wT_B = work.tile([128, 512], BF, tag="wT_B", bufs=4)
                    nc.scalar.activation(wT_A, sAB[:, 0:512], EXP, scale=SCALE)
                    nc.scalar.activation(wT_B, sAB[:, 512:1024], EXP, scale=SCALE)
                    j = kt - 4 * qb
                    if j >= 0:  # diagonal supertile: causal select
                        ncols = 128 * j + 128
                        for wT in (wT_A, wT_B):
                            nc.gpsimd.affine_select(
                                out=wT[:, 0:ncols],
                                in_=wT[:, 0:ncols],
                                compare_op=mybir.AluOpType.is_ge,
                                fill=0.0,
                                base=-128 * j,
                                pattern=[[1, ncols]],
                                channel_multiplier=-1,
                            )
                    nc.tensor.matmul(
                        av_A, V[:, kt, hA, :], wT_A,
                        start=(kt == 0), stop=(kt == nkt - 1),
                    )
                    nc.tensor.matmul(
                        av_B, V[:, kt, hB, :], wT_B,
                        start=(kt == 0), stop=(kt == nkt - 1),
                    )
                # stage accumulators off PSUM, normalize off-critical-path
                avc = work.tile([65, 1024], F32, tag="avc", bufs=2, name="avc")
                nc.vector.tensor_copy(avc[:, 0:512], av_A)
                nc.vector.tensor_copy(avc[:, 512:1024], av_B)
                rec = work.tile([65, 1024], F32, tag="rec", bufs=2, name="rec")
                nc.vector.reciprocal(rec[64:65, :], avc[64:65, :])
                rec_d = dpool.tile([1, 1024], F32, tag="rec_d", bufs=4,
                                   name="rec_d")
                nc.sync.dma_start(out=rec_d, in_=rec[64:65, :])
                rep = work.tile([64, 1024], F32, tag="rep", bufs=2, name="rep")
                nc.sync.dma_start(
                    out=rep,
                    in_=bass.AP(rec_d.tensor, rec_d.offset, [[0, 64], [1, 1024]]),
                )
                att = work.tile([128, 512], BF, tag=f"attTq{g}", bufs=2,
                                name=f"attTq{g}")
                nc.vector.tensor_mul(att[0:64, :], avc[0:64, 0:512], rep[:, 0:512])
                tmpB = work.tile([64, 512], BF, tag="tmpB", bufs=2, name="tmpB")
                nc.vector.tensor_mul(tmpB, avc[0:64, 512:1024], rep[:, 512:1024])
                nc.sync.dma_start(out=att[64:128, :], in_=tmpB)
                attTq.append(att)

            # ---- out projection for this quarter's q rows ----
            for qtl in range(4):
                qt = rnd * 4 + qtl
                psy = ps.tile([128, 1024], F32, tag="pp", name="psy")
                for g in range(NG):
                    for half in range(2):
                        nc.tensor.matmul(
                            psy[:, half * 512:(half + 1) * 512],
                            attTq[g][:, qtl * 128:(qtl + 1) * 128],
                            wo_bf[:, g, half * 512:(half + 1) * 512],
                            start=(g == 0),
                            stop=(g == NG - 1),
                        )
                y_sb = work.tile([128, C], F32, tag="y_sb", bufs=2, name="y_sb")
                nc.vector.tensor_copy(y_sb, psy)
                nc.sync.dma_start(
                    out=y_d.ap()[qt * 128:(qt + 1) * 128, :], in_=y_sb
                )

    nc.compile()
    return nc


_NC_CACHE = None


def _get_nc():
    global _NC_CACHE
    if _NC_CACHE is None:
        _NC_CACHE = build_nc()
    return _NC_CACHE


def kernel(x, w_qkv, w_out, _trace=False):
    B = x.shape[0]
    x = np.ascontiguousarray(x, dtype=np.float32)
    w_qkv = np.ascontiguousarray(w_qkv, dtype=np.float32)
    w_out = np.ascontiguousarray(w_out, dtype=np.float32)

    nc = _get_nc()
    in_maps = []
    for core in range(8):
        b = core % B
        hbase = (core // B) * HC
        lo, hi = hbase * D, hbase * D + HC * D
        in_maps.append({
            "x": x[b],
            "wq": np.ascontiguousarray(w_qkv[:, lo:hi]),
            "wk": np.ascontiguousarray(w_qkv[:, C + lo:C + hi]),
            "wv": np.ascontiguousarray(w_qkv[:, 2 * C + lo:2 * C + hi]),
            "wo": np.ascontiguousarray(w_out[lo:hi, :]),
        })

    res = run_bass_kernel_spmd(nc, in_maps, core_ids=list(range(8)), trace=_trace)
    ys = [r["y"] for r in res.results]
    out = np.empty((B, T, C), dtype=np.float32)
    for b in range(B):
        out[b] = ys[b] + ys[b + B]
    if _trace:
        return out, res
    return out


# revision 17
# speedup vs baseline: 1.1546x; 1.0471x over previous
"""Causal self-attention for trn2, 8 NeuronCores.

Problem: x[4,2048,1024] @ w_qkv[1024,3072] -> causal MHA (16 heads, d=64)
-> @ w_out[1024,1024].

Sharding: core c handles batch b=c%4 and heads hbase=8*(c//4)..hbase+8
(data parallel on B x tensor parallel on heads). Each core computes the
partial out-projection y_c = att_slice @ w_out[slice]; the host sums the
two partials per batch.

v4: all matmul operands bf16 (fp32 PSUM accumulation). x is cast to a
ct-major bf16 DRAM scratch (SWDGE cast-DMA, contiguous [2048,128] blocks)
and transposed with hardware DMA-transpose loads. All weights are cast
once into resident bf16 tiles by SWDGE cast-DMAs. Softmax denominators
come from a fused ones-column in the AV matmul ([V|1]^T w^T row 64);
causal masking skips above-diagonal tiles and applies one gpsimd
affine_select per diagonal 128x128 block after the exp. Normalization:
DVE reciprocal + DRAM-bounce partition broadcast + multiply, staged off
PSUM so nothing blocks the accumulators.

4-round pipeline over T-quarters: round r transposes quarter r, projects
qT/kT/V for it, runs attention q-block r for every head (causality needs
only k/V quarters <= r), then the out-projection for those q rows. PSUM:
sA/sB double-buffered [128,512] scores, av_A/av_B accumulators, and a
dedicated [128,1024] projection tag so next-round projection matmuls can
fill TensorE gaps while ScalarE paces the attention exps.
"""

import sys

for p in ("/opt/trn_rl_repo", "/opt/pypackages"):
    if p not in sys.path:
        sys.path.insert(0, p)

import contextlib

import numpy as np

import concourse.bass as bass
import concourse.mybir as mybir
import concourse.tile as tile
from concourse import bacc
from concourse.bass_utils import run_bass_kernel_spmd

F32 = mybir.dt.float32
BF = mybir.dt.bfloat16
EXP = mybir.ActivationFunctionType.Exp

T = 2048          # sequence length
C = 1024          # model dim
HC = 8            # heads per core
D = 64            # head dim
NG = 4            # head-groups of 2 per core
NCT = C // 128    # 8 contraction tiles
NTT = T // 128    # 16 token tiles
SCALE = 0.125     # 1/sqrt(D)


def build_nc():
    nc = bacc.Bacc("TRN2", target_bir_lowering=False, debug=False)

    x_d = nc.dram_tensor("x", [T, C], F32, kind="ExternalInput")
    wq_d = nc.dram_tensor("wq", [C, 512], F32, kind="ExternalInput")
    wk_d = nc.dram_tensor("wk", [C, 512], F32, kind="ExternalInput")
    wv_d = nc.dram_tensor("wv", [C, 512], F32, kind="ExternalInput")
    wo_d = nc.dram_tensor("wo", [512, C], F32, kind="ExternalInput")
    y_d = nc.dram_tensor("y", [T, C], F32, kind="ExternalOutput")

    with tile.TileContext(nc) as tc, contextlib.ExitStack() as ctx:
        persist = ctx.enter_context(tc.tile_pool(name="persist", bufs=1))
        work = ctx.enter_context(tc.tile_pool(name="work", bufs=1))
        ps = ctx.enter_context(tc.tile_pool(name="ps", bufs=1, space="PSUM"))
        dpool = ctx.enter_context(tc.tile_pool(name="dram", bufs=1, space="DRAM"))

        kT = [persist.tile([128, T], BF, tag=f"kT{g}", name=f"kT{g}")
              for g in range(NG)]
        V = persist.tile([128, NTT, HC, 65], BF, tag="V")

        # x -> bf16 DRAM scratch. The cast must be a CONTIGUOUS SWDGE DMA:
        # strided cast-DMAs truncate instead of round-to-nearest, and the
        # truncation bias blows up the dot products downstream.
        xbf = dpool.tile([T, C], BF, tag="xbf", name="xbf")
        for rnd in range(4):
            nc.gpsimd.dma_start(
                out=xbf[rnd * 512:(rnd + 1) * 512, :],
                in_=x_d.ap()[rnd * 512:(rnd + 1) * 512, :],
            )

        # resident bf16 weights: contiguous SWDGE cast to DRAM (exact RNE),
        # then plain rearranged bf16 loads
        wqd_bf = dpool.tile([C, 512], BF, tag="wqd_bf", name="wqd_bf")
        nc.gpsimd.dma_start(out=wqd_bf, in_=wq_d.ap())
        wkd_bf = dpool.tile([C, 512], BF, tag="wkd_bf", name="wkd_bf")
        nc.gpsimd.dma_start(out=wkd_bf, in_=wk_d.ap())
        wvd_bf = dpool.tile([C, 512], BF, tag="wvd_bf", name="wvd_bf")
        nc.gpsimd.dma_start(out=wvd_bf, in_=wv_d.ap())
        wod_bf = dpool.tile([512, C], BF, tag="wod_bf", name="wod_bf")
        nc.gpsimd.dma_start(out=wod_bf, in_=wo_d.ap())
        wq_bf = persist.tile([128, NCT, 512], BF, tag="wq_bf")
        nc.sync.dma_start(
            out=wq_bf, in_=wqd_bf.rearrange("(ct p) m -> p ct m", p=128))
        wk_bf = persist.tile([128, NCT, 512], BF, tag="wk_bf")
        nc.sync.dma_start(
            out=wk_bf, in_=wkd_bf.rearrange("(ct p) m -> p ct m", p=128))
        wv_bf = persist.tile([128, NCT, 512], BF, tag="wv_bf")
        nc.sync.dma_start(
            out=wv_bf, in_=wvd_bf.rearrange("(ct p) m -> p ct m", p=128))
        wo_bf = persist.tile([128, NG, C], BF, tag="wo_bf")
        nc.sync.dma_start(
            out=wo_bf, in_=wod_bf.rearrange("(g p) c -> p g c", p=128))

        # ones column of V
        ones_f32 = persist.tile([128, NTT, HC], F32, tag="ones")
        nc.vector.memset(ones_f32, 1.0)
        nc.vector.tensor_copy(V[:, :, :, 64], ones_f32)

        for rnd in range(4):
            q0 = rnd * 512  # first token of this quarter

            # ---- xT quarter via hardware DMA-transpose (2 HWDGE rings) ----
            xTq = [work.tile([128, 512], BF, tag=f"xTq{ct}", name=f"xTq{ct}",
                             bufs=2)
                   for ct in range(NCT)]
            for ct in range(NCT):
                nc.sync.dma_start_transpose(
                    out=xTq[ct], in_=xbf[q0:q0 + 512, ct * 128:(ct + 1) * 128]
                )

            # ---- qT/kT for this quarter ----
            qTq = []
            for g in range(NG):
                pqk = ps.tile([128, 1024], F32, tag="pp", name="pqk")
                for ct in range(NCT):
                    nc.tensor.matmul(
                        pqk[:, 0:512],
                        wq_bf[:, ct, g * 128:(g + 1) * 128],
                        xTq[ct],
                        start=(ct == 0), stop=(ct == NCT - 1),
                    )
                    nc.tensor.matmul(
                        pqk[:, 512:1024],
                        wk_bf[:, ct, g * 128:(g + 1) * 128],
                        xTq[ct],
                        start=(ct == 0), stop=(ct == NCT - 1),
                    )
                qq = work.tile([128, 512], BF, tag=f"qTq{g}", bufs=2,
                               name=f"qTq{g}")
                nc.vector.tensor_copy(qq, pqk[:, 0:512])
                qTq.append(qq)
                nc.vector.tensor_copy(kT[g][:, q0:q0 + 512], pqk[:, 512:1024])

            # ---- V for this quarter (two tt-pairs per psum tile) ----
            for half in range(2):
                pv = ps.tile([128, 1024], F32, tag="pp", name="pv")
                for ct in range(NCT):
                    for sub in range(2):
                        jl = half * 2 + sub
                        nc.tensor.matmul(
                            pv[:, sub * 512:(sub + 1) * 512],
                            xTq[ct][:, jl * 128:(jl + 1) * 128],
                            wv_bf[:, ct, :],
                            start=(ct == 0), stop=(ct == NCT - 1),
                        )
                for sub in range(2):
                    tt = rnd * 4 + half * 2 + sub
                    for h in range(HC):
                        nc.vector.tensor_copy(
                            V[:, tt, h, 0:64],
                            pv[:, sub * 512 + h * 64: sub * 512 + h * 64 + 64],
                        )

            # ---- attention: q-block rnd for every group ----
            # Heads sequential, 2-kt score batches: 2-matmul bursts into a
            # [128,1024] psum span, one exp, causal select on diagonal
            # blocks, then a 2-matmul AV burst.
            qb = rnd
            nkt = 4 * (qb + 1)
            attTq = []
            for g in range(NG):
                att = work.tile([128, 512], BF, tag=f"attTq{g}", bufs=2,
                                name=f"attTq{g}")
                for hh in range(2):
                    head = 2 * g + hh
                    r0, r1 = 64 * hh, 64 * hh + 64
                    tp = (64 * hh, 0)
                    av = ps.tile([65, 512], F32, tag=f"av{hh}", name="av")
                    for b0 in range(0, nkt, 2):
                        sc = ps.tile([128, 1024], F32, tag="sc", bufs=2, name="sc")
                        for m in range(2):
                            nc.tensor.matmul(
                                sc[:, m * 512:(m + 1) * 512],
                                kT[g][r0:r1, (b0 + m) * 128:(b0 + m + 1) * 128],
                                qTq[g][r0:r1, :],
                                start=True, stop=True,
                                tile_position=tp,
                            )
                        wT = work.tile([128, 1024], BF, tag="wT", bufs=3)
                        nc.scalar.activation(wT, sc, EXP, scale=SCALE)
                        for m in range(2):
                            j = b0 + m - 4 * qb
                            if j >= 0:  # diagonal 128-block: causal select
                                ncols = 128 * j + 128
                                nc.gpsimd.affine_select(
                                    out=wT[:, m * 512:m * 512 + ncols],
                                    in_=wT[:, m * 512:m * 512 + ncols],
                                    compare_op=mybir.AluOpType.is_ge,
                                    fill=0.0,
                                    base=-128 * j,
                                    pattern=[[1, ncols]],
                                    channel_multiplier=-1,
                                )
                        for m in range(2):
                            kt = b0 + m
                            nc.tensor.matmul(
                                av, V[:, kt, head, :],
                                wT[:, m * 512:(m + 1) * 512],
                                start=(kt == 0), stop=(kt == nkt - 1),
                            )
                    # stage off PSUM, normalize off the critical path
                    avc = work.tile([65, 512], F32, tag="avc", bufs=4, name="avc")
                    nc.vector.tensor_copy(avc, av)
                    rec = work.tile([65, 512], F32, tag="rec", bufs=4, name="rec")
                    nc.vector.reciprocal(rec[64:65, :], avc[64:65, :])
                    rec_d = dpool.tile([1, 512], F32, tag="rec_d", bufs=4,
                                       name="rec_d")
                    nc.sync.dma_start(out=rec_d, in_=rec[64:65, :])
                    rep = work.tile([64, 512], F32, tag="rep", bufs=4, name="rep")
                    nc.sync.dma_start(
                        out=rep,
                        in_=bass.AP(rec_d.tensor, rec_d.offset,
                                    [[0, 64], [1, 512]]),
                    )
                    if hh == 0:
                        nc.vector.tensor_mul(att[0:64, :], avc[0:64, :], rep)
                    else:
                        tmpB = work.tile([64, 512], BF, tag="tmpB", bufs=2,
                                         name="tmpB")
                        nc.vector.tensor_mul(tmpB, avc[0:64, :], rep)
                        nc.sync.dma_start(out=att[64:128, :], in_=tmpB)
                attTq.append(att)

            # ---- out projection for this quarter's q rows ----
            for qtl in range(4):
                qt = rnd * 4 + qtl
                psy = ps.tile([128, 1024], F32, tag="pp", name="psy")
                for g in range(NG):
                    for half in range(2):
                        nc.tensor.matmul(
                            psy[:, half * 512:(half + 1) * 512],
                            attTq[g][:, qtl * 128:(qtl + 1) * 128],
                            wo_bf[:, g, half * 512:(half + 1) * 512],
                            start=(g == 0),
                            stop=(g == NG - 1),
                        )
                y_sb = work.tile([128, C], F32, tag="y_sb", bufs=2, name="y_sb")
                nc.vector.tensor_copy(y_sb, psy)
                nc.sync.dma_start(
                    out=y_d.ap()[qt * 128:(qt + 1) * 128, :], in_=y_sb
                )

    nc.compile()
    return nc


_NC_CACHE = None


def _get_nc():
    global _NC_CACHE
    if _NC_CACHE is None:
        _NC_CACHE = build_nc()
    return _NC_CACHE


def kernel(x, w_qkv, w_out, _trace=False):
    B = x.shape[0]
    x = np.ascontiguousarray(x, dtype=np.float32)
    w_qkv = np.ascontiguousarray(w_qkv, dtype=np.float32)
    w_out = np.ascontiguousarray(w_out, dtype=np.float32)

    nc = _get_nc()
    in_maps = []
    for core in range(8):
        b = core % B
        hbase = (core // B) * HC
        lo, hi = hbase * D, hbase * D + HC * D
        in_maps.append({
            "x": x[b],
            "wq": np.ascontiguousarray(w_qkv[:, lo:hi]),
            "wk": np.ascontiguousarray(w_qkv[:, C + lo:C + hi]),
            "wv": np.ascontiguousarray(w_qkv[:, 2 * C + lo:2 * C + hi]),
            "wo": np.ascontiguousarray(w_out[lo:hi, :]),
        })

    res = run_bass_kernel_spmd(nc, in_maps, core_ids=list(range(8)), trace=_trace)
    ys = [r["y"] for r in res.results]
    out = np.empty((B, T, C), dtype=np.float32)
    for b in range(B):
        out[b] = ys[b] + ys[b + B]
    if _trace:
        return out, res
    return out


# revision 18
# speedup vs baseline: 1.1690x; 1.0125x over previous
"""Causal self-attention for trn2, 8 NeuronCores.

Problem: x[4,2048,1024] @ w_qkv[1024,3072] -> causal MHA (16 heads, d=64)
-> @ w_out[1024,1024].

Sharding: core c handles batch b=c%4 and heads hbase=8*(c//4)..hbase+8
(data parallel on B x tensor parallel on heads). Each core computes the
partial out-projection y_c = att_slice @ w_out[slice]; the host sums the
two partials per batch.

v4: all matmul operands bf16 (fp32 PSUM accumulation). x is cast to a
ct-major bf16 DRAM scratch (SWDGE cast-DMA, contiguous [2048,128] blocks)
and transposed with hardware DMA-transpose loads. All weights are cast
once into resident bf16 tiles by SWDGE cast-DMAs. Softmax denominators
come from a fused ones-column in the AV matmul ([V|1]^T w^T row 64);
causal masking skips above-diagonal tiles and applies one gpsimd
affine_select per diagonal 128x128 block after the exp. Normalization:
DVE reciprocal + DRAM-bounce partition broadcast + multiply, staged off
PSUM so nothing blocks the accumulators.

4-round pipeline over T-quarters: round r transposes quarter r, projects
qT/kT/V for it, runs attention q-block r for every head (causality needs
only k/V quarters <= r), then the out-projection for those q rows. PSUM:
sA/sB double-buffered [128,512] scores, av_A/av_B accumulators, and a
dedicated [128,1024] projection tag so next-round projection matmuls can
fill TensorE gaps while ScalarE paces the attention exps.
"""

import sys

for p in ("/opt/trn_rl_repo", "/opt/pypackages"):
    if p not in sys.path:
        sys.path.insert(0, p)

import contextlib

import numpy as np

import concourse.bass as bass
import concourse.mybir as mybir
import concourse.tile as tile
from concourse import bacc
from concourse.bass_utils import run_bass_kernel_spmd

F32 = mybir.dt.float32
BF = mybir.dt.bfloat16
EXP = mybir.ActivationFunctionType.Exp

T = 2048          # sequence length
C = 1024          # model dim
HC = 8            # heads per core
D = 64            # head dim
NG = 4            # head-groups of 2 per core
NCT = C // 128    # 8 contraction tiles
NTT = T // 128    # 16 token tiles
SCALE = 0.125     # 1/sqrt(D)


def build_nc():
    nc = bacc.Bacc("TRN2", target_bir_lowering=False, debug=False)

    x_d = nc.dram_tensor("x", [T, C], F32, kind="ExternalInput")
    wq_d = nc.dram_tensor("wq", [C, 512], F32, kind="ExternalInput")
    wk_d = nc.dram_tensor("wk", [C, 512], F32, kind="ExternalInput")
    wv_d = nc.dram_tensor("wv", [C, 512], F32, kind="ExternalInput")
    wo_d = nc.dram_tensor("wo", [512, C], F32, kind="ExternalInput")
    y_d = nc.dram_tensor("y", [T, C], F32, kind="ExternalOutput")

    with tile.TileContext(nc) as tc, contextlib.ExitStack() as ctx:
        persist = ctx.enter_context(tc.tile_pool(name="persist", bufs=1))
        work = ctx.enter_context(tc.tile_pool(name="work", bufs=1))
        ps = ctx.enter_context(tc.tile_pool(name="ps", bufs=1, space="PSUM"))
        dpool = ctx.enter_context(tc.tile_pool(name="dram", bufs=1, space="DRAM"))

        kT = [persist.tile([128, T], BF, tag=f"kT{g}", name=f"kT{g}")
              for g in range(NG)]
        V = persist.tile([128, NTT, HC, 65], BF, tag="V")

        # x -> bf16 DRAM scratch. The cast must be a CONTIGUOUS SWDGE DMA:
        # strided cast-DMAs truncate instead of round-to-nearest, and the
        # truncation bias blows up the dot products downstream.
        xbf = dpool.tile([T, C], BF, tag="xbf", name="xbf")
        nc.gpsimd.dma_start(out=xbf[0:512, :], in_=x_d.ap()[0:512, :])
        # round-0 transposes issued before the weight loads so the sync ring
        # delivers them first and projection can start early
        xTq0 = [work.tile([128, 512], BF, tag=f"xTq{ct}", name=f"xTq{ct}",
                          bufs=2)
                for ct in range(NCT)]
        for ct in range(NCT):
            nc.sync.dma_start_transpose(
                out=xTq0[ct], in_=xbf[0:512, ct * 128:(ct + 1) * 128]
            )
        for rnd in range(1, 4):
            nc.gpsimd.dma_start(
                out=xbf[rnd * 512:(rnd + 1) * 512, :],
                in_=x_d.ap()[rnd * 512:(rnd + 1) * 512, :],
            )

        # resident bf16 weights: contiguous SWDGE cast to DRAM (exact RNE),
        # then plain rearranged bf16 loads
        wqd_bf = dpool.tile([C, 512], BF, tag="wqd_bf", name="wqd_bf")
        nc.gpsimd.dma_start(out=wqd_bf, in_=wq_d.ap())
        wkd_bf = dpool.tile([C, 512], BF, tag="wkd_bf", name="wkd_bf")
        nc.gpsimd.dma_start(out=wkd_bf, in_=wk_d.ap())
        wvd_bf = dpool.tile([C, 512], BF, tag="wvd_bf", name="wvd_bf")
        nc.gpsimd.dma_start(out=wvd_bf, in_=wv_d.ap())
        wod_bf = dpool.tile([512, C], BF, tag="wod_bf", name="wod_bf")
        nc.gpsimd.dma_start(out=wod_bf, in_=wo_d.ap())
        wq_bf = persist.tile([128, NCT, 512], BF, tag="wq_bf")
        nc.sync.dma_start(
            out=wq_bf, in_=wqd_bf.rearrange("(ct p) m -> p ct m", p=128))
        wk_bf = persist.tile([128, NCT, 512], BF, tag="wk_bf")
        nc.sync.dma_start(
            out=wk_bf, in_=wkd_bf.rearrange("(ct p) m -> p ct m", p=128))
        wv_bf = persist.tile([128, NCT, 512], BF, tag="wv_bf")
        nc.sync.dma_start(
            out=wv_bf, in_=wvd_bf.rearrange("(ct p) m -> p ct m", p=128))
        wo_bf = persist.tile([128, NG, C], BF, tag="wo_bf")
        nc.sync.dma_start(
            out=wo_bf, in_=wod_bf.rearrange("(g p) c -> p g c", p=128))

        # ones column of V
        ones_f32 = persist.tile([128, NTT, HC], F32, tag="ones")
        nc.vector.memset(ones_f32, 1.0)
        nc.vector.tensor_copy(V[:, :, :, 64], ones_f32)

        for rnd in range(4):
            q0 = rnd * 512  # first token of this quarter

            # ---- xT quarter via hardware DMA-transpose ----
            if rnd == 0:
                xTq = xTq0
            else:
                xTq = [work.tile([128, 512], BF, tag=f"xTq{ct}",
                                 name=f"xTq{ct}", bufs=2)
                       for ct in range(NCT)]
                for ct in range(NCT):
                    nc.sync.dma_start_transpose(
                        out=xTq[ct],
                        in_=xbf[q0:q0 + 512, ct * 128:(ct + 1) * 128]
                    )

            # ---- qT/kT for this quarter ----
            qTq = []
            for g in range(NG):
                pqk = ps.tile([128, 1024], F32, tag="pp", name="pqk")
                for ct in range(NCT):
                    nc.tensor.matmul(
                        pqk[:, 0:512],
                        wq_bf[:, ct, g * 128:(g + 1) * 128],
                        xTq[ct],
                        start=(ct == 0), stop=(ct == NCT - 1),
                    )
                    nc.tensor.matmul(
                        pqk[:, 512:1024],
                        wk_bf[:, ct, g * 128:(g + 1) * 128],
                        xTq[ct],
                        start=(ct == 0), stop=(ct == NCT - 1),
                    )
                qq = work.tile([128, 512], BF, tag=f"qTq{g}", bufs=2,
                               name=f"qTq{g}")
                nc.vector.tensor_copy(qq, pqk[:, 0:512])
                qTq.append(qq)
                nc.vector.tensor_copy(kT[g][:, q0:q0 + 512], pqk[:, 512:1024])

            # ---- V for this quarter (two tt-pairs per psum tile) ----
            for half in range(2):
                pv = ps.tile([128, 1024], F32, tag="pp", name="pv")
                for ct in range(NCT):
                    for sub in range(2):
                        jl = half * 2 + sub
                        nc.tensor.matmul(
                            pv[:, sub * 512:(sub + 1) * 512],
                            xTq[ct][:, jl * 128:(jl + 1) * 128],
                            wv_bf[:, ct, :],
                            start=(ct == 0), stop=(ct == NCT - 1),
                        )
                for sub in range(2):
                    tt = rnd * 4 + half * 2 + sub
                    for h in range(HC):
                        nc.vector.tensor_copy(
                            V[:, tt, h, 0:64],
                            pv[:, sub * 512 + h * 64: sub * 512 + h * 64 + 64],
                        )

            # ---- attention: q-block rnd for every group ----
            # Heads sequential, 2-kt score batches: 2-matmul bursts into a
            # [128,1024] psum span, one exp, causal select on diagonal
            # blocks, then a 2-matmul AV burst.
            qb = rnd
            nkt = 4 * (qb + 1)
            attTq = []
            for g in range(NG):
                att = work.tile([128, 512], BF, tag=f"attTq{g}", bufs=2,
                                name=f"attTq{g}")
                for hh in range(2):
                    head = 2 * g + hh
                    r0, r1 = 64 * hh, 64 * hh + 64
                    tp = (64 * hh, 0)
                    av = ps.tile([65, 512], F32, tag=f"av{hh}", name="av")
                    for b0 in range(0, nkt, 2):
                        sc = ps.tile([128, 1024], F32, tag="sc", bufs=2, name="sc")
                        for m in range(2):
                            nc.tensor.matmul(
                                sc[:, m * 512:(m + 1) * 512],
                                kT[g][r0:r1, (b0 + m) * 128:(b0 + m + 1) * 128],
                                qTq[g][r0:r1, :],
                                start=True, stop=True,
                                tile_position=tp,
                            )
                        wT = work.tile([128, 1024], BF, tag="wT", bufs=3)
                        nc.scalar.activation(wT, sc, EXP, scale=SCALE)
                        for m in range(2):
                            j = b0 + m - 4 * qb
                            if j >= 0:  # diagonal 128-block: causal select
                                ncols = 128 * j + 128
                                nc.gpsimd.affine_select(
                                    out=wT[:, m * 512:m * 512 + ncols],
                                    in_=wT[:, m * 512:m * 512 + ncols],
                                    compare_op=mybir.AluOpType.is_ge,
                                    fill=0.0,
                                    base=-128 * j,
                                    pattern=[[1, ncols]],
                                    channel_multiplier=-1,
                                )
                        for m in range(2):
                            kt = b0 + m
                            nc.tensor.matmul(
                                av, V[:, kt, head, :],
                                wT[:, m * 512:(m + 1) * 512],
                                start=(kt == 0), stop=(kt == nkt - 1),
                            )
                    # stage off PSUM, normalize off the critical path
                    avc = work.tile([65, 512], F32, tag="avc", bufs=4, name="avc")
                    nc.vector.tensor_copy(avc, av)
                    rec = work.tile([65, 512], F32, tag="rec", bufs=4, name="rec")
                    nc.vector.reciprocal(rec[64:65, :], avc[64:65, :])
                    rec_d = dpool.tile([1, 512], F32, tag="rec_d", bufs=4,
                                       name="rec_d")
                    nc.sync.dma_start(out=rec_d, in_=rec[64:65, :])
                    rep = work.tile([64, 512], F32, tag="rep", bufs=4, name="rep")
                    nc.sync.dma_start(
                        out=rep,
                        in_=bass.AP(rec_d.tensor, rec_d.offset,
                                    [[0, 64], [1, 512]]),
                    )
                    if hh == 0:
                        nc.vector.tensor_mul(att[0:64, :], avc[0:64, :], rep)
                    else:
                        tmpB = work.tile([64, 512], BF, tag="tmpB", bufs=2,
                                         name="tmpB")
                        nc.vector.tensor_mul(tmpB, avc[0:64, :], rep)
                        nc.sync.dma_start(out=att[64:128, :], in_=tmpB)
                attTq.append(att)

            # ---- out projection for this quarter's q rows ----
            for qtl in range(4):
                qt = rnd * 4 + qtl
                psy = ps.tile([128, 1024], F32, tag="pp", name="psy")
                for g in range(NG):
                    for half in range(2):
                        nc.tensor.matmul(
                            psy[:, half * 512:(half + 1) * 512],
                            attTq[g][:, qtl * 128:(qtl + 1) * 128],
                            wo_bf[:, g, half * 512:(half + 1) * 512],
                            start=(g == 0),
                            stop=(g == NG - 1),
                        )
                y_sb = work.tile([128, C], F32, tag="y_sb", bufs=2, name="y_sb")
                nc.vector.tensor_copy(y_sb, psy)
                nc.sync.dma_start(
                    out=y_d.ap()[qt * 128:(qt + 1) * 128, :], in_=y_sb
                )

    nc.compile()
    return nc


_NC_CACHE = None


def _get_nc():
    global _NC_CACHE
    if _NC_CACHE is None:
        _NC_CACHE = build_nc()
    return _NC_CACHE


def kernel(x, w_qkv, w_out, _trace=False):
    B = x.shape[0]
    x = np.ascontiguousarray(x, dtype=np.float32)
    w_qkv = np.ascontiguousarray(w_qkv, dtype=np.float32)
    w_out = np.ascontiguousarray(w_out, dtype=np.float32)

    nc = _get_nc()
    in_maps = []
    for core in range(8):
        b = core % B
        hbase = (core // B) * HC
        lo, hi = hbase * D, hbase * D + HC * D
        in_maps.append({
            "x": x[b],
            "wq": np.ascontiguousarray(w_qkv[:, lo:hi]),
            "wk": np.ascontiguousarray(w_qkv[:, C + lo:C + hi]),
            "wv": np.ascontiguousarray(w_qkv[:, 2 * C + lo:2 * C + hi]),
            "wo": np.ascontiguousarray(w_out[lo:hi, :]),
        })

    res = run_bass_kernel_spmd(nc, in_maps, core_ids=list(range(8)), trace=_trace)
    ys = [r["y"] for r in res.results]
    out = np.empty((B, T, C), dtype=np.float32)
    for b in range(B):
        out[b] = ys[b] + ys[b + B]
    if _trace:
        return out, res
    return out


# revision 19
# speedup vs baseline: 1.2012x; 1.0276x over previous
"""Causal self-attention for trn2, 8 NeuronCores.

Problem: x[4,2048,1024] @ w_qkv[1024,3072] -> causal MHA (16 heads, d=64)
-> @ w_out[1024,1024].

Sharding: core c handles batch b=c%4 and heads hbase=8*(c//4)..hbase+8
(data parallel on B x tensor parallel on heads). Each core computes the
partial out-projection y_c = att_slice @ w_out[slice]; the host sums the
two partials per batch.

v4: all matmul operands bf16 (fp32 PSUM accumulation). x is cast to a
ct-major bf16 DRAM scratch (SWDGE cast-DMA, contiguous [2048,128] blocks)
and transposed with hardware DMA-transpose loads. All weights are cast
once into resident bf16 tiles by SWDGE cast-DMAs. Softmax denominators
come from a fused ones-column in the AV matmul ([V|1]^T w^T row 64);
causal masking skips above-diagonal tiles and applies one gpsimd
affine_select per diagonal 128x128 block after the exp. Normalization:
DVE reciprocal + DRAM-bounce partition broadcast + multiply, staged off
PSUM so nothing blocks the accumulators.

4-round pipeline over T-quarters: round r transposes quarter r, projects
qT/kT/V for it, runs attention q-block r for every head (causality needs
only k/V quarters <= r), then the out-projection for those q rows. PSUM:
sA/sB double-buffered [128,512] scores, av_A/av_B accumulators, and a
dedicated [128,1024] projection tag so next-round projection matmuls can
fill TensorE gaps while ScalarE paces the attention exps.
"""

import sys

for p in ("/opt/trn_rl_repo", "/opt/pypackages"):
    if p not in sys.path:
        sys.path.insert(0, p)

import contextlib

import numpy as np

import concourse.bass as bass
import concourse.mybir as mybir
import concourse.tile as tile
from concourse import bacc
from concourse.bass_utils import run_bass_kernel_spmd

F32 = mybir.dt.float32
BF = mybir.dt.bfloat16
EXP = mybir.ActivationFunctionType.Exp

T = 2048          # sequence length
C = 1024          # model dim
HC = 8            # heads per core
D = 64            # head dim
NG = 4            # head-groups of 2 per core
NCT = C // 128    # 8 contraction tiles
NTT = T // 128    # 16 token tiles
SCALE = 0.125     # 1/sqrt(D)


def build_nc():
    nc = bacc.Bacc("TRN2", target_bir_lowering=False, debug=False)

    x_d = nc.dram_tensor("x", [T, C], F32, kind="ExternalInput")
    wq_d = nc.dram_tensor("wq", [C, 512], F32, kind="ExternalInput")
    wk_d = nc.dram_tensor("wk", [C, 512], F32, kind="ExternalInput")
    wv_d = nc.dram_tensor("wv", [C, 512], F32, kind="ExternalInput")
    wo_d = nc.dram_tensor("wo", [512, C], F32, kind="ExternalInput")
    y_d = nc.dram_tensor("y", [T, C], F32, kind="ExternalOutput")

    with tile.TileContext(nc) as tc, contextlib.ExitStack() as ctx:
        persist = ctx.enter_context(tc.tile_pool(name="persist", bufs=1))
        work = ctx.enter_context(tc.tile_pool(name="work", bufs=1))
        ps = ctx.enter_context(tc.tile_pool(name="ps", bufs=1, space="PSUM"))
        dpool = ctx.enter_context(tc.tile_pool(name="dram", bufs=1, space="DRAM"))

        kT = [persist.tile([128, T], BF, tag=f"kT{g}", name=f"kT{g}")
              for g in range(NG)]
        V = persist.tile([128, NTT, HC, 65], BF, tag="V")

        # x -> bf16 DRAM scratch. The cast must be a CONTIGUOUS SWDGE DMA:
        # strided cast-DMAs truncate instead of round-to-nearest, and the
        # truncation bias blows up the dot products downstream.
        xbf = dpool.tile([T, C], BF, tag="xbf", name="xbf")
        nc.gpsimd.dma_start(out=xbf[0:512, :], in_=x_d.ap()[0:512, :])
        # round-0 transposes issued before the weight loads so the sync ring
        # delivers them first and projection can start early
        xTq0 = [work.tile([128, 512], BF, tag=f"xTq{ct}", name=f"xTq{ct}",
                          bufs=2)
                for ct in range(NCT)]
        for ct in range(NCT):
            nc.sync.dma_start_transpose(
                out=xTq0[ct], in_=xbf[0:512, ct * 128:(ct + 1) * 128]
            )
        # resident bf16 weights: contiguous SWDGE cast to DRAM (exact RNE),
        # then plain rearranged bf16 loads. Emitted before the quarter 1-3
        # x casts so round 0's projection isn't stuck behind them.
        wqd_bf = dpool.tile([C, 512], BF, tag="wqd_bf", name="wqd_bf")
        nc.gpsimd.dma_start(out=wqd_bf, in_=wq_d.ap())
        wkd_bf = dpool.tile([C, 512], BF, tag="wkd_bf", name="wkd_bf")
        nc.gpsimd.dma_start(out=wkd_bf, in_=wk_d.ap())
        wvd_bf = dpool.tile([C, 512], BF, tag="wvd_bf", name="wvd_bf")
        nc.gpsimd.dma_start(out=wvd_bf, in_=wv_d.ap())
        wod_bf = dpool.tile([512, C], BF, tag="wod_bf", name="wod_bf")
        nc.gpsimd.dma_start(out=wod_bf, in_=wo_d.ap())
        wq_bf = persist.tile([128, NCT, 512], BF, tag="wq_bf")
        nc.sync.dma_start(
            out=wq_bf, in_=wqd_bf.rearrange("(ct p) m -> p ct m", p=128))
        wk_bf = persist.tile([128, NCT, 512], BF, tag="wk_bf")
        nc.sync.dma_start(
            out=wk_bf, in_=wkd_bf.rearrange("(ct p) m -> p ct m", p=128))
        wv_bf = persist.tile([128, NCT, 512], BF, tag="wv_bf")
        nc.sync.dma_start(
            out=wv_bf, in_=wvd_bf.rearrange("(ct p) m -> p ct m", p=128))
        wo_bf = persist.tile([128, NG, C], BF, tag="wo_bf")
        nc.sync.dma_start(
            out=wo_bf, in_=wod_bf.rearrange("(g p) c -> p g c", p=128))

        for rnd in range(1, 4):
            nc.gpsimd.dma_start(
                out=xbf[rnd * 512:(rnd + 1) * 512, :],
                in_=x_d.ap()[rnd * 512:(rnd + 1) * 512, :],
            )

        # ones column of V
        ones_f32 = persist.tile([128, NTT, HC], F32, tag="ones")
        nc.vector.memset(ones_f32, 1.0)
        nc.vector.tensor_copy(V[:, :, :, 64], ones_f32)

        for rnd in range(4):
            q0 = rnd * 512  # first token of this quarter

            # ---- xT quarter via hardware DMA-transpose ----
            if rnd == 0:
                xTq = xTq0
            else:
                xTq = [work.tile([128, 512], BF, tag=f"xTq{ct}",
                                 name=f"xTq{ct}", bufs=2)
                       for ct in range(NCT)]
                for ct in range(NCT):
                    nc.sync.dma_start_transpose(
                        out=xTq[ct],
                        in_=xbf[q0:q0 + 512, ct * 128:(ct + 1) * 128]
                    )

            # ---- qT/kT for this quarter ----
            qTq = []
            for g in range(NG):
                pqk = ps.tile([128, 1024], F32, tag="pp", name="pqk")
                for ct in range(NCT):
                    nc.tensor.matmul(
                        pqk[:, 0:512],
                        wq_bf[:, ct, g * 128:(g + 1) * 128],
                        xTq[ct],
                        start=(ct == 0), stop=(ct == NCT - 1),
                    )
                    nc.tensor.matmul(
                        pqk[:, 512:1024],
                        wk_bf[:, ct, g * 128:(g + 1) * 128],
                        xTq[ct],
                        start=(ct == 0), stop=(ct == NCT - 1),
                    )
                qq = work.tile([128, 512], BF, tag=f"qTq{g}", bufs=2,
                               name=f"qTq{g}")
                nc.vector.tensor_copy(qq, pqk[:, 0:512])
                qTq.append(qq)
                nc.vector.tensor_copy(kT[g][:, q0:q0 + 512], pqk[:, 512:1024])

            # ---- V for this quarter (two tt-pairs per psum tile) ----
            for half in range(2):
                pv = ps.tile([128, 1024], F32, tag="pp", name="pv")
                for ct in range(NCT):
                    for sub in range(2):
                        jl = half * 2 + sub
                        nc.tensor.matmul(
                            pv[:, sub * 512:(sub + 1) * 512],
                            xTq[ct][:, jl * 128:(jl + 1) * 128],
                            wv_bf[:, ct, :],
                            start=(ct == 0), stop=(ct == NCT - 1),
                        )
                for sub in range(2):
                    tt = rnd * 4 + half * 2 + sub
                    for h in range(HC):
                        nc.vector.tensor_copy(
                            V[:, tt, h, 0:64],
                            pv[:, sub * 512 + h * 64: sub * 512 + h * 64 + 64],
                        )

            # ---- attention: q-block rnd for every group ----
            # Heads sequential, 2-kt score batches: 2-matmul bursts into a
            # [128,1024] psum span, one exp, causal select on diagonal
            # blocks, then a 2-matmul AV burst.
            qb = rnd
            nkt = 4 * (qb + 1)
            attTq = []
            for g in range(NG):
                att = work.tile([128, 512], BF, tag=f"attTq{g}", bufs=2,
                                name=f"attTq{g}")
                for hh in range(2):
                    head = 2 * g + hh
                    r0, r1 = 64 * hh, 64 * hh + 64
                    tp = (64 * hh, 0)
                    av = ps.tile([65, 512], F32, tag=f"av{hh}", name="av")
                    for b0 in range(0, nkt, 2):
                        sc = ps.tile([128, 1024], F32, tag="sc", bufs=2, name="sc")
                        for m in range(2):
                            nc.tensor.matmul(
                                sc[:, m * 512:(m + 1) * 512],
                                kT[g][r0:r1, (b0 + m) * 128:(b0 + m + 1) * 128],
                                qTq[g][r0:r1, :],
                                start=True, stop=True,
                                tile_position=tp,
                            )
                        wT = work.tile([128, 1024], BF, tag="wT", bufs=3)
                        nc.scalar.activation(wT, sc, EXP, scale=SCALE)
                        for m in range(2):
                            j = b0 + m - 4 * qb
                            if j >= 0:  # diagonal 128-block: causal select
                                ncols = 128 * j + 128
                                nc.gpsimd.affine_select(
                                    out=wT[:, m * 512:m * 512 + ncols],
                                    in_=wT[:, m * 512:m * 512 + ncols],
                                    compare_op=mybir.AluOpType.is_ge,
                                    fill=0.0,
                                    base=-128 * j,
                                    pattern=[[1, ncols]],
                                    channel_multiplier=-1,
                                )
                        for m in range(2):
                            kt = b0 + m
                            nc.tensor.matmul(
                                av, V[:, kt, head, :],
                                wT[:, m * 512:(m + 1) * 512],
                                start=(kt == 0), stop=(kt == nkt - 1),
                            )
                    # stage off PSUM, normalize off the critical path
                    avc = work.tile([65, 512], F32, tag="avc", bufs=4, name="avc")
                    nc.vector.tensor_copy(avc, av)
                    rec = work.tile([65, 512], F32, tag="rec", bufs=4, name="rec")
                    nc.vector.reciprocal(rec[64:65, :], avc[64:65, :])
                    rec_d = dpool.tile([1, 512], F32, tag="rec_d", bufs=4,
                                       name="rec_d")
                    nc.sync.dma_start(out=rec_d, in_=rec[64:65, :])
                    rep = work.tile([64, 512], F32, tag="rep", bufs=4, name="rep")
                    nc.sync.dma_start(
                        out=rep,
                        in_=bass.AP(rec_d.tensor, rec_d.offset,
                                    [[0, 64], [1, 512]]),
                    )
                    if hh == 0:
                        nc.vector.tensor_mul(att[0:64, :], avc[0:64, :], rep)
                    else:
                        tmpB = work.tile([64, 512], BF, tag="tmpB", bufs=2,
                                         name="tmpB")
                        nc.vector.tensor_mul(tmpB, avc[0:64, :], rep)
                        nc.sync.dma_start(out=att[64:128, :], in_=tmpB)
                attTq.append(att)

            # ---- out projection for this quarter's q rows ----
            for qtl in range(4):
                qt = rnd * 4 + qtl
                psy = ps.tile([128, 1024], F32, tag="pp", name="psy")
                for g in range(NG):
                    for half in range(2):
                        nc.tensor.matmul(
                            psy[:, half * 512:(half + 1) * 512],
                            attTq[g][:, qtl * 128:(qtl + 1) * 128],
                            wo_bf[:, g, half * 512:(half + 1) * 512],
                            start=(g == 0),
                            stop=(g == NG - 1),
                        )
                y_sb = work.tile([128, C], F32, tag="y_sb", bufs=2, name="y_sb")
                nc.vector.tensor_copy(y_sb, psy)
                nc.sync.dma_start(
                    out=y_d.ap()[qt * 128:(qt + 1) * 128, :], in_=y_sb
                )

    nc.compile()
    return nc


_NC_CACHE = None


def _get_nc():
    global _NC_CACHE
    if _NC_CACHE is None:
        _NC_CACHE = build_nc()
    return _NC_CACHE


def kernel(x, w_qkv, w_out, _trace=False):
    B = x.shape[0]
    x = np.ascontiguousarray(x, dtype=np.float32)
    w_qkv = np.ascontiguousarray(w_qkv, dtype=np.float32)
    w_out = np.ascontiguousarray(w_out, dtype=np.float32)

    nc = _get_nc()
    in_maps = []
    for core in range(8):
        b = core % B
        hbase = (core // B) * HC
        lo, hi = hbase * D, hbase * D + HC * D
        in_maps.append({
            "x": x[b],
            "wq": np.ascontiguousarray(w_qkv[:, lo:hi]),
            "wk": np.ascontiguousarray(w_qkv[:, C + lo:C + hi]),
            "wv": np.ascontiguousarray(w_qkv[:, 2 * C + lo:2 * C + hi]),
            "wo": np.ascontiguousarray(w_out[lo:hi, :]),
        })

    res = run_bass_kernel_spmd(nc, in_maps, core_ids=list(range(8)), trace=_trace)
    ys = [r["y"] for r in res.results]
    out = np.empty((B, T, C), dtype=np.float32)
    for b in range(B):
        out[b] = ys[b] + ys[b + B]
    if _trace:
        return out, res
    return out
